# revision 6
# baseline (speedup 1.0000x reference)
"""Trainium2 Bass kernel for nn_FLossNoSoftMax (topk_masking).

Computes  -sum_b mean_v[(1-mask)*log(1-x)]  where mask marks the top-c
entries per row of x [2048, 50257] f32.

Math: per row  loss_b = (S_b - T_b)/V  with
  S_b = sum_v log(1-x[b,v])
  T_b = sum over the c largest values m of log(1-m)   (multiset, tie-exact)
result = -sum_b loss_b.

Device kernel (per core, 256 rows = 2 blocks of 128 partitions): stream
row-chunks via SWDGE (gpsimd) loads — the gpsimd ring sustains the
contended ~365 GB/s HBM rate gaplessly; HWDGE bursts higher only when
the other cores' NCs are idle.  Scalar engine computes Ln(1-x) with fused
per-partition accumulation (-> S); vector engine computes per-chunk top-8
values (InstMax); chunk top-8s merge with a final InstMax giving the
exact global top-8 multiset per row, whose first c entries yield T.

Critical-path design:
- The first two loads of block 0 go via HWDGE (nc.sync) which has a
  faster first-byte path, starting the stream ~1.5us earlier; the bulk
  goes via gpsimd so the sync ring stays empty for the result stores
  (a store waiting on compute must never sit ahead of loads in a DMA
  ring's FIFO — that head-of-line block cost 31us in an earlier rev).
- Chunk sizes taper geometrically at the block end (1536/768/384/192/81)
  so ACT and DVE — both slower per element than the stream — finish
  each chunk before the remaining stream ends; only the 81-wide tail
  compute runs after the last byte lands.
- Per-block results store separately: block 0's HBM write receipt hides
  under block 1's stream.
- Teardown (see _fast_teardown): no all-engine barriers of our own;
  GpSimd alone waits for everything (incl. the final store receipt),
  resets DGE state, and releases DVE; Sync/Scalar/PE run ahead into the
  NEFF wrapper epilogue.

Output: per-row (S_b - T_b); host does the final -sum/V in float64.
Sharding: data-parallel over the batch dim, 256 rows per core on 8 cores.
"""

import sys

sys.path.insert(0, "/opt/trn_rl_repo")

import numpy as np

from concourse import bacc, bass, mybir, tile
from concourse.bass_utils import run_bass_kernel_spmd
from concourse.vector_clock import ScopedClock


def _ensure_axon_hooks():
    """The agent image lacks antenv.axon_hooks; run_bass_kernel_spmd imports
    it when tracing is requested (e.g. BASS_TRACE=1). Provide the module and
    wire the ctypes NTFF hook so tracing works instead of crashing."""
    try:
        import antenv.axon_hooks  # noqa: F401

        return
    except ImportError:
        pass
    import types

    try:
        import antenv
    except ImportError:
        return
    mod = types.ModuleType("antenv.axon_hooks")
    store = {"h": None}
    mod.set_axon_ntff_profile_hook = lambda h: store.__setitem__("h", h)
    mod.get_axon_ntff_profile_hook = lambda: store.get("h")
    sys.modules["antenv.axon_hooks"] = mod
    antenv.axon_hooks = mod
    try:
        from trn_agent_boot.trn_boot import _ntff_profile_via_ctypes

        mod.set_axon_ntff_profile_hook(
            _ntff_profile_via_ctypes("/opt/axon/libaxon_pjrt.so")
        )
        from concourse import bass_utils as _bu

        _bu.upload_artifacts = lambda d: "local://" + d
    except Exception:
        pass


_ensure_axon_hooks()


def _fast_teardown(self, tick_clock, wait_clock):
    # Replaces Tile's stock drain + 2x all-engine-barrier tail.  The NEFF
    # wrapper's epilogue (an all-engine barrier, then each engine serially
    # clearing a ~50-semaphore slice of S[3..255], then a final barrier)
    # starts once the slowest engine exits our code — so the teardown here
    # avoids gating fast engines on slow ones.  GpSimd alone waits for all
    # outstanding body work (including the final store's HBM write
    # receipt), resets DGE state + clears the tile sems, then releases
    # Vector; Sync/Scalar/PE proceed straight to the wrapper barrier.
    nc = self.nc
    gp = nc.gpsimd.engine
    # Cheap pipeline drains on everything but GpSimd (a GpSimd drain is a
    # dge_drain, which is expensive).
    for eng_type, eng in nc.engines.items():
        if eng_type == gp:
            continue
        d = mybir.InstDrain(
            name=nc.get_next_instruction_name(), ins=[], outs=[],
            bass_is_fusable=False,
        )
        d.engine = eng_type
        eng.add_instruction(d)

    popped = nc._tile_sem_poison_stack.pop()
    assert popped is self._sem_poison

    rel = nc.alloc_semaphore("teardown_release")

    # GpSimd: wait for all outstanding body work (the attached sem waits
    # include every DMA's completion), then reset + clear the tile sems.
    sems = list(self.sems.allocated().values())
    sem_nums = [
        s.num if isinstance(s, bass.SemaphoreHandle) else s for s in sems
    ]
    first = True
    for sem_range in bass.compact_to_ranges(sem_nums):
        assert nc._state.free_isdisjoint(sem_range)
        r = nc.gpsimd.dma_reset(sem_range)
        if first:
            wait_clock.add_sem_waits(
                r.ins, ScopedClock({None: tick_clock.global_clock})
            )
            first = False
        nc.gpsimd.sem_clear(sem_range)
    nc._state.prepend_free_semaphores(sem_nums)
    for poison_set in nc._tile_sem_poison_stack:
        poison_set.update(sem_nums)

    nc.gpsimd.sem_inc(rel, 1)
    nc.vector.wait_ge(rel, 1)
    # Explicit clear so a second NEFF execution starts from 0 even if the
    # wrapper flood's coverage of `rel` shifts.
    nc.vector.sem_clear(range(rel.num, rel.num + 1))


tile.TileContext._drain_and_barrier = _fast_teardown

B, V = 2048, 50257
N_CORES = 8
ROWS_PER_CORE = B // N_CORES  # 256
P = 128
BLOCKS = ROWS_PER_CORE // P  # 2
F = 3072
# Block 0: plain layout; its end-of-block compute hides under block 1's
# stream.  Block 1: geometric taper so only the 192-wide final chunk's
# compute runs after the stream ends.
CHUNKS0 = [F] * 16 + [1105]  # 49152 + 1105
CHUNKS1 = [1297] + [F] * 15 + [1536, 768, 384, 192]  # 1297+46080+2880
assert sum(CHUNKS0) == V and sum(CHUNKS1) == V

f32 = mybir.dt.float32
Ln = mybir.ActivationFunctionType.Ln
AX = mybir.AxisListType.X

_cache: dict = {}


def _build(top_c: int) -> bass.Bass:
    nc = bacc.Bacc("TRN2", target_bir_lowering=False)
    x = nc.dram_tensor("x", [ROWS_PER_CORE, V], f32, kind="ExternalInput")
    # out[blk, p] = S - T for row blk*128 + p
    out = nc.dram_tensor("out", [BLOCKS, P], f32, kind="ExternalOutput")

    def _offsets(sizes):
        off, out = 0, []
        for sz in sizes:
            out.append((off, sz))
            off += sz
        return out

    blk_chunks = [_offsets(CHUNKS0), _offsets(CHUNKS1)]

    with tile.TileContext(nc) as tc:
        with (
            tc.tile_pool(name="xp", bufs=10) as xp,
            tc.tile_pool(name="xsp", bufs=1) as xsp,
            tc.tile_pool(name="yp", bufs=1) as yp,
            tc.tile_pool(name="st", bufs=1) as st,
        ):
            # DVE-initialized bias tile: keeps the activation-bias const off
            # the Pool-engine prologue, which delays the first load descgen.
            bias_t = st.tile([P, 1], f32, tag="bias_t")
            nc.vector.memset(bias_t[:], 1.0)
            yt = yp.tile([P, F], f32, tag="yt")
            for blk in range(BLOCKS):
                rows = slice(blk * P, (blk + 1) * P)
                chunks = blk_chunks[blk]
                nhead = len(chunks) - 1
                s_parts = st.tile([P, nhead], f32, tag=f"s_parts{blk}")
                top8s = st.tile([P, 8 * nhead], f32, tag=f"top8s{blk}")
                top16 = st.tile([P, 16], f32, tag=f"top16_{blk}")
                for c, (coff, sz) in enumerate(chunks[:-1]):
                    if sz == F:
                        xt = xp.tile([P, sz], f32, tag="xt")
                    else:
                        xt = xsp.tile([P, sz], f32, tag=f"xs{blk}_{sz}")
                    # First two loads of the run go HWDGE: faster first
                    # byte, and the sync ring is otherwise unused until
                    # the stores.
                    dma_eng = nc.sync if (blk == 0 and c < 2) else nc.gpsimd
                    dma_eng.dma_start(out=xt[:], in_=x[rows, coff : coff + sz])
                    nc.scalar.activation(
                        yt[:, :sz],
                        xt[:],
                        Ln,
                        bias=bias_t[:, 0:1],
                        scale=-1.0,
                        accum_out=s_parts[:, c : c + 1],
                    )
                    nc.vector.max(top8s[:, 8 * c : 8 * (c + 1)], xt[:])
                # pre-merge + pre-reduce of the streamed chunks — issued
                # before the last chunk so they run while it is in flight
                nc.vector.max(top16[:, 0:8], top8s[:])
                s_a = st.tile([P, 1], f32, tag=f"s_a{blk}")
                nc.vector.reduce_sum(s_a[:], s_parts[:], axis=AX)
                # last chunk — the only compute after its bytes land
                loff, lsz = chunks[-1]
                xr = xsp.tile([P, lsz], f32, tag=f"xl{blk}")
                nc.gpsimd.dma_start(out=xr[:], in_=x[rows, loff : loff + lsz])
                s_last = st.tile([P, 1], f32, tag=f"s_last{blk}")
                nc.scalar.activation(
                    yt[:, :lsz],
                    xr[:],
                    Ln,
                    bias=bias_t[:, 0:1],
                    scale=-1.0,
                    accum_out=s_last[:],
                )
                nc.vector.max(top16[:, 8:16], xr[:])
                # final merge + T
                m8f = st.tile([P, 8], f32, tag=f"m8f{blk}")
                nc.vector.max(m8f[:], top16[:])
                lnm = st.tile([P, top_c], f32, tag=f"lnm{blk}")
                t_sum = st.tile([P, 1], f32, tag=f"t_sum{blk}")
                nc.scalar.activation(
                    lnm[:], m8f[:, :top_c], Ln, bias=bias_t[:, 0:1],
                    scale=-1.0, accum_out=t_sum[:],
                )
                s_tot = st.tile([P, 1], f32, tag=f"s_tot{blk}")
                nc.vector.tensor_add(s_tot[:], s_a[:], s_last[:])
                res = st.tile([P, 1], f32, tag=f"res{blk}")
                nc.vector.tensor_sub(res[:], s_tot[:], t_sum[:])
                # store this block's 512B as soon as it is ready; block 0's
                # HBM write receipt hides under block 1's stream.  Stores
                # ride the sync ring, which carries no pending loads.
                nc.sync.dma_start(out=out[blk, :], in_=res[:, 0])
    nc.compile()
    return nc


def _get(top_c: int) -> bass.Bass:
    if top_c not in _cache:
        _cache[top_c] = _build(top_c)
    return _cache[top_c]


def _run(output: np.ndarray, top_c: int, **spmd_kwargs):
    assert 1 <= top_c <= 8, f"kernel supports top_c in [1,8], got {top_c}"
    x = np.ascontiguousarray(np.asarray(output, dtype=np.float32))
    assert x.shape == (B, V), x.shape
    nc = _get(top_c)
    in_maps = [
        {"x": x[i * ROWS_PER_CORE : (i + 1) * ROWS_PER_CORE]} for i in range(N_CORES)
    ]
    res = run_bass_kernel_spmd(nc, in_maps, list(range(N_CORES)), **spmd_kwargs)
    parts = np.concatenate([r["out"].reshape(-1) for r in res.results])
    total = -np.sum(parts.astype(np.float64)) / V
    return np.float32(total), res


def kernel(top_c, output) -> np.ndarray:
    val, _ = _run(output, int(top_c))
    return np.array(val, dtype=np.float32)
